# revision 13
# baseline (speedup 1.0000x reference)
"""CrossViT fused block on 8 TRN2 NeuronCores.

Sharding: 2 branches (vis-output / ir-output) x 4-way token split -> 8 cores,
no collectives. Each core computes 49 output tokens of one branch end-to-end:
LN1, cross-attention (its queries vs all 196 keys/values of the other
modality), projection, residual, LN2, FFN, residual. Activations are kept
feature-major (transposed: features on SBUF partitions) so every linear layer
is matmul(lhsT=W_natural, rhs=xT). Matmul operands are bf16 (fp32 PSUM
accumulation); LayerNorm / softmax statistics in fp32.

Host side: shards + head-major-reorders weights, transposes embeddings,
gathers the 8 (256, 49) outputs, and applies the output pixel-shuffle.
"""
import sys
if '/opt/trn_rl_repo' not in sys.path:
    sys.path.insert(0, '/opt/trn_rl_repo')

import numpy as np
import ml_dtypes

BF = ml_dtypes.bfloat16
N, EMB, H, DH, HID = 196, 256, 8, 32, 1024
T = 49            # tokens per core
EPS, SCALE = 1e-5, 16.0
P = 128
NCORES = 8
TOKC = ((0, 128), (128, 68))   # token chunks of the 196 keys/values

_CACHE = {}


# ---------------------------------------------------------------- bass build
def build_bass():
    import concourse.bacc as bacc
    import concourse.mybir as mybir
    import concourse.tile as tile

    f32 = mybir.dt.float32
    bf16 = mybir.dt.bfloat16
    AF = mybir.ActivationFunctionType
    OP = mybir.AluOpType

    nc = bacc.Bacc("TRN2", target_bir_lowering=False)

    # ---- DRAM I/O (identical names on every core; data differs per core)
    xq_d = nc.dram_tensor("xq", [EMB, T], f32, kind="ExternalInput")
    xkv_d = nc.dram_tensor("xkv", [EMB, N], bf16, kind="ExternalInput")
    wq_d = nc.dram_tensor("wq", [EMB, EMB], bf16, kind="ExternalInput")
    wk_d = nc.dram_tensor("wk", [EMB, EMB], bf16, kind="ExternalInput")
    wv_d = nc.dram_tensor("wv", [EMB, EMB], bf16, kind="ExternalInput")
    wp_d = nc.dram_tensor("wp", [EMB, EMB], bf16, kind="ExternalInput")
    w1_d = nc.dram_tensor("w1", [EMB, HID], bf16, kind="ExternalInput")
    w2_d = nc.dram_tensor("w2", [HID, EMB], bf16, kind="ExternalInput")
    bq_d = nc.dram_tensor("bq", [EMB], f32, kind="ExternalInput")
    bk_d = nc.dram_tensor("bk", [EMB], f32, kind="ExternalInput")
    bv_d = nc.dram_tensor("bv", [1, EMB], f32, kind="ExternalInput")
    bp_d = nc.dram_tensor("bp", [EMB], f32, kind="ExternalInput")
    b1_d = nc.dram_tensor("b1", [HID], f32, kind="ExternalInput")
    b2_d = nc.dram_tensor("b2", [EMB], f32, kind="ExternalInput")
    ln1w_d = nc.dram_tensor("ln1w", [EMB], f32, kind="ExternalInput")
    ln1b_d = nc.dram_tensor("ln1b", [EMB], f32, kind="ExternalInput")
    ln2w_d = nc.dram_tensor("ln2w", [EMB], f32, kind="ExternalInput")
    ln2b_d = nc.dram_tensor("ln2b", [EMB], f32, kind="ExternalInput")
    out_d = nc.dram_tensor("out", [EMB, T], f32, kind="ExternalOutput")

    id98_d = nc.inline_tensor(np.eye(98, dtype=BF), name="id98c")

    with tile.TileContext(nc) as tc:
        with (
            tc.tile_pool(name="const", bufs=1) as cpool,
            tc.tile_pool(name="act", bufs=1) as apool,
            tc.tile_pool(name="attp", bufs=4) as attpool,
            tc.tile_pool(name="ps_mm", bufs=2, space="PSUM") as ps_mm,
            tc.tile_pool(name="ps_s", bufs=2, space="PSUM") as ps_s,
            tc.tile_pool(name="ps_t", bufs=2, space="PSUM") as ps_t,
            tc.tile_pool(name="ps_o", bufs=1, space="PSUM") as ps_o,
            tc.tile_pool(name="ps_ln", bufs=1, space="PSUM") as ps_ln,
        ):
            # ---------------- input DMAs
            def load(dram_ap, shape, dt, pool=cpool, tag=None):
                t = pool.tile(shape, dt, tag=tag or dram_ap.tensor.name)
                nc.sync.dma_start(t[:], dram_ap)
                return t

            xq_sb = load(xq_d.rearrange("(c p) t -> p c t", p=P), [P, 2, T], f32)
            xkv_bf = load(xkv_d.rearrange("(c p) t -> p c t", p=P), [P, 2, N], bf16)
            wq_sb = load(wq_d.rearrange("(c p) m -> p c m", p=P), [P, 2, EMB], bf16)
            wk_sb = load(wk_d.rearrange("(c p) m -> p c m", p=P), [P, 2, EMB], bf16)
            wv_sb = load(wv_d.rearrange("(c p) m -> p c m", p=P), [P, 2, EMB], bf16)
            wp_sb = load(wp_d.rearrange("(c p) m -> p c m", p=P), [P, 2, EMB], bf16)
            w1_sb = load(w1_d.rearrange("(c p) m -> p c m", p=P), [P, 2, HID], bf16)
            w2_sb = load(w2_d.rearrange("(c p) m -> p c m", p=P), [P, 8, EMB], bf16)
            bq_sb = load(bq_d.rearrange("(c p) -> p c", p=64), [64, 4], f32)
            bk_sb = load(bk_d.rearrange("(c p) -> p c", p=64), [64, 4], f32)
            bp_sb = load(bp_d.rearrange("(c p) -> p c", p=P), [P, 2], f32)
            b1_sb = load(b1_d.rearrange("(c p) -> p c", p=P), [P, 8], f32)
            b2_sb = load(b2_d.rearrange("(c p) -> p c", p=P), [P, 2], f32)
            ln1w_sb = load(ln1w_d.rearrange("(c p) -> p c", p=P), [P, 2], f32)
            ln1b_sb = load(ln1b_d.rearrange("(c p) -> p c", p=P), [P, 2], f32)
            ln2w_sb = load(ln2w_d.rearrange("(c p) -> p c", p=P), [P, 2], f32)
            ln2b_sb = load(ln2b_d.rearrange("(c p) -> p c", p=P), [P, 2], f32)
            bv_bc = load(bv_d[:, :].to_broadcast((P, EMB)), [P, EMB], f32)
            id98 = load(id98_d[:, :], [98, 98], bf16)

            ones_bf = cpool.tile([P, 1], bf16)
            nc.vector.memset(ones_bf[:], 1.0)
            ones1f = cpool.tile([1, P], f32)
            nc.vector.memset(ones1f[:], 1.0)
            eps_sb = cpool.tile([1, 1], f32)
            nc.vector.memset(eps_sb[:], EPS)

            # bf16 copy of xq for matmuls
            xq_bf = apool.tile([P, 2, T], bf16)
            nc.vector.tensor_copy(xq_bf[:], xq_sb[:])

            # ---------------- layernorm (feature-major; features on partitions)
            # PSUM accumulation groups are tracked per 2KB bank, so each LN
            # uses exactly two groups with a data dependency between them:
            # one stats matmul (sum x | sum x^2) and one broadcast matmul.
            def layer_norm(x_f32, x_bf, w_sb, b_sb, out_f32, tag):
                lnp = ps_ln.tile([P, 512], f32, tag="ln")
                xsq = apool.tile([P, 2, 2 * T], bf16, tag=f"xsq_{tag}")
                nc.vector.tensor_copy(xsq[:, :, 0:T], x_bf[:])
                nc.vector.tensor_tensor(xsq[:, :, T:2 * T], x_bf[:], x_bf[:],
                                        op=OP.mult)
                for kc in range(2):
                    nc.tensor.matmul(lnp[0:1, 0:2 * T], ones_bf[:], xsq[:, kc],
                                     start=(kc == 0), stop=(kc == 1))
                # mr: [0:T] mean, [T:2T] rstd
                mr = apool.tile([1, 2 * T], f32, tag=f"mr_{tag}")
                nc.scalar.mul(mr[0:1, 0:T], lnp[0:1, 0:T], 1.0 / EMB)
                msq = apool.tile([1, T], f32, tag=f"msq_{tag}")
                nc.scalar.mul(msq[:], lnp[0:1, T:2 * T], 1.0 / EMB)
                var = apool.tile([1, T], f32, tag=f"var_{tag}")
                nc.vector.tensor_tensor(var[:], mr[0:1, 0:T], mr[0:1, 0:T],
                                        op=OP.mult)
                nc.vector.tensor_tensor(var[:], msq[:], var[:], op=OP.subtract)
                std = apool.tile([1, T], f32, tag=f"std_{tag}")
                nc.scalar.activation(std[:], var[:], AF.Sqrt, bias=eps_sb[:])
                nc.vector.reciprocal(mr[0:1, T:2 * T], std[:])
                # broadcast mean|rstd over 128 partitions via one K=1 fp32 matmul
                nc.tensor.matmul(lnp[:, 2 * T:4 * T], ones1f[:], mr[:],
                                 start=True, stop=True)
                tmp = apool.tile([P, 2, T], f32, tag=f"lntmp_{tag}")
                for kc in range(2):
                    nc.vector.tensor_tensor(tmp[:, kc], x_f32[:, kc],
                                            lnp[:, 2 * T:3 * T], op=OP.subtract)
                    nc.vector.tensor_tensor(tmp[:, kc], tmp[:, kc],
                                            lnp[:, 3 * T:4 * T], op=OP.mult)
                    nc.vector.tensor_scalar(out_f32[:, kc], tmp[:, kc],
                                            w_sb[:, kc:kc + 1], b_sb[:, kc:kc + 1],
                                            op0=OP.mult, op1=OP.add)

            # ---------------- q (block-diagonal per head pair), k (pair-major), v
            # bd[pair]: (64, 98) bf16, [[q_h0^T (32,49), 0], [0, q_h1^T (32,49)]]
            bd_tiles = []
            for pr in range(4):
                bd = attpool.tile([64, 2 * T], bf16, tag="bd")
                nc.vector.memset(bd[:], 0.0)
                for j in range(2):
                    h = 2 * pr + j
                    pbd = ps_t.tile([64, 512], f32, tag="small")
                    for kc in range(2):
                        nc.tensor.matmul(pbd[j * DH:(j + 1) * DH, j * T:(j + 1) * T],
                                         wq_sb[:, kc, h * DH:(h + 1) * DH],
                                         xq_bf[:, kc], start=(kc == 0), stop=(kc == 1))
                    nc.scalar.activation(bd[j * DH:(j + 1) * DH, j * T:(j + 1) * T],
                                         pbd[j * DH:(j + 1) * DH, j * T:(j + 1) * T],
                                         AF.Identity,
                                         bias=bq_sb[j * DH:(j + 1) * DH, pr:pr + 1])
                bd_tiles.append(bd)

            # k pair-major: (64, 4, N); pair p rows = heads (2p, 2p+1) features
            k_bf = apool.tile([64, 4, N], bf16)
            for pr in range(4):
                pk = ps_mm.tile([P, 512], f32, tag="mm")
                for kc in range(2):
                    nc.tensor.matmul(pk[:64], wk_sb[:, kc, pr * 64:(pr + 1) * 64],
                                     xkv_bf[:, kc], start=(kc == 0), stop=(kc == 1))
                nc.scalar.activation(k_bf[:, pr], pk[:64], AF.Identity,
                                     bias=bk_sb[:, pr:pr + 1])

            v_bf = []
            for tcx, (t0, tsz) in enumerate(TOKC):
                pv = ps_mm.tile([P, 512], f32, tag="mm")
                for kc in range(2):
                    nc.tensor.matmul(pv[:tsz], xkv_bf[:, kc, t0:t0 + tsz],
                                     wv_sb[:, kc], start=(kc == 0), stop=(kc == 1))
                vt = apool.tile([P, EMB], bf16, tag=f"v{tcx}")
                nc.vector.tensor_tensor(vt[:tsz], pv[:tsz], bv_bc[:tsz], op=OP.add)
                v_bf.append(vt)

            # ---------------- scores + softmax; rows of each tile = 2 heads x 49 q
            ssum = apool.tile([2 * T, 4], f32, tag="ssum")
            att_tiles = []
            for pr in range(4):
                pss = ps_s.tile([2 * T, 512], f32, tag="scores")
                nc.tensor.matmul(pss[:], bd_tiles[pr][:], k_bf[:, pr],
                                 start=True, stop=True)
                att = attpool.tile([2 * T, N], bf16, tag="att")
                nc.scalar.activation(att[:], pss[:], AF.Exp, scale=1.0 / SCALE,
                                     accum_out=ssum[:, pr:pr + 1])
                att_tiles.append(att)

            sinv = apool.tile([2 * T, 4], f32, tag="sinv")
            nc.vector.reciprocal(sinv[:], ssum[:])
            for pr in range(4):
                nc.vector.tensor_scalar(att_tiles[pr][:], att_tiles[pr][:],
                                        sinv[:, pr:pr + 1], None, op0=OP.mult)

            # ---------------- attT via PE transpose, then o^T_h = v_h^T @ att_h^T
            po = ps_o.tile([P, 512], f32, tag="oT")
            for pr in range(4):
                ats = []
                for tcx, (t0, tsz) in enumerate(TOKC):
                    pt = ps_t.tile([P, 1024], bf16, tag="small")
                    nc.tensor.transpose(pt[:tsz, 0:2 * T],
                                        att_tiles[pr][:, t0:t0 + tsz], id98[:])
                    at = attpool.tile([P, 2 * T], bf16, tag="attT")
                    nc.vector.tensor_copy(at[:tsz], pt[:tsz, 0:2 * T])
                    ats.append(at)
                # per-head accumulation groups, strictly one open group per bank
                for j in range(2):
                    h = 2 * pr + j
                    mc, prow = h // 4, (h % 4) * DH
                    for tcx, (t0, tsz) in enumerate(TOKC):
                        nc.tensor.matmul(po[prow:prow + DH, mc * T:(mc + 1) * T],
                                         v_bf[tcx][:tsz, h * DH:(h + 1) * DH],
                                         ats[tcx][:tsz, j * T:(j + 1) * T],
                                         start=(tcx == 0), stop=(tcx == 1),
                                         tile_position=(0, prow))

            oT_bf = apool.tile([P, 2, T], bf16)
            for mc in range(2):
                nc.vector.tensor_copy(oT_bf[:, mc], po[:, mc * T:(mc + 1) * T])

            # ---------------- LN1 (needed only now, for the residual)
            nvT = apool.tile([P, 2, T], f32, tag="nvT")
            layer_norm(xq_sb, xq_bf, ln1w_sb, ln1b_sb, nvT, "ln1")

            # ---------------- projection + residual
            rv = apool.tile([P, 2, T], f32, tag="rv")
            for mc in range(2):
                pp = ps_mm.tile([P, 512], f32, tag="mm")
                for kc in range(2):
                    nc.tensor.matmul(pp[:], wp_sb[:, kc, mc * P:(mc + 1) * P],
                                     oT_bf[:, kc], start=(kc == 0), stop=(kc == 1))
                nc.vector.tensor_scalar(rv[:, mc], pp[:], bp_sb[:, mc:mc + 1], None,
                                        op0=OP.add)
                nc.vector.tensor_tensor(rv[:, mc], rv[:, mc], nvT[:, mc], op=OP.add)
            rv_bf = apool.tile([P, 2, T], bf16, tag="rvbf")
            nc.vector.tensor_copy(rv_bf[:], rv[:])

            # ---------------- LN2
            lv = apool.tile([P, 2, T], f32, tag="lv")
            layer_norm(rv, rv_bf, ln2w_sb, ln2b_sb, lv, "ln2")
            lv_bf = apool.tile([P, 2, T], bf16, tag="lvbf")
            nc.vector.tensor_copy(lv_bf[:], lv[:])

            # ---------------- FFN + residual
            g_bf = apool.tile([P, 8, T], bf16, tag="gelu")
            for mc in range(8):
                ph = ps_mm.tile([P, 512], f32, tag="mm")
                for kc in range(2):
                    nc.tensor.matmul(ph[:], w1_sb[:, kc, mc * P:(mc + 1) * P],
                                     lv_bf[:, kc], start=(kc == 0), stop=(kc == 1))
                nc.scalar.activation(g_bf[:, mc], ph[:], AF.Gelu,
                                     bias=b1_sb[:, mc:mc + 1])

            out_sb = apool.tile([P, 2, T], f32, tag="out")
            for mc in range(2):
                pf = ps_mm.tile([P, 512], f32, tag="mm")
                for kc in range(8):
                    nc.tensor.matmul(pf[:], w2_sb[:, kc, mc * P:(mc + 1) * P],
                                     g_bf[:, kc], start=(kc == 0), stop=(kc == 7))
                nc.vector.tensor_scalar(out_sb[:, mc], pf[:], b2_sb[:, mc:mc + 1],
                                        None, op0=OP.add)
                nc.vector.tensor_tensor(out_sb[:, mc], out_sb[:, mc], lv[:, mc],
                                        op=OP.add)

            nc.sync.dma_start(out_d.rearrange("(c p) t -> p c t", p=P), out_sb[:])

    nc.compile()
    return nc


# ---------------------------------------------------------------- host side
def _reorder_qkv(W, b):
    W4 = np.asarray(W, np.float32).reshape(EMB, H, DH, 3)
    b4 = np.asarray(b, np.float32).reshape(H, DH, 3)
    return ([np.ascontiguousarray(W4[:, :, :, i].reshape(EMB, EMB)) for i in range(3)],
            [np.ascontiguousarray(b4[:, :, i].reshape(EMB)) for i in range(3)])


def make_in_maps(inputs):
    inp = {k: np.asarray(v, np.float32) for k, v in inputs.items()}
    qkv_v = _reorder_qkv(inp['Wqkv_v'], inp['bqkv_v'])
    qkv_i = _reorder_qkv(inp['Wqkv_i'], inp['bqkv_i'])
    maps = []
    for core in range(NCORES):
        branch = core // 4
        r0 = (core % 4) * T
        if branch == 0:   # vis output: vis queries, ir keys/values
            x_own, x_oth = inp['vis_emb'][0], inp['ir_emb'][0]
            (wq, bq), (wk, bk), (wv, bv) = \
                (qkv_v[0][0], qkv_v[1][0]), (qkv_i[0][1], qkv_i[1][1]), \
                (qkv_i[0][2], qkv_i[1][2])
            wp, bp = inp['Wp_v'], inp['bp_v']
            lnw = (inp['ln1v_w'], inp['ln1v_b'], inp['ln2v_w'], inp['ln2v_b'])
            w1, b1, w2, b2 = inp['W1v'], inp['b1v'], inp['W2v'], inp['b2v']
        else:             # ir output: ir queries, vis keys/values
            x_own, x_oth = inp['ir_emb'][0], inp['vis_emb'][0]
            (wq, bq), (wk, bk), (wv, bv) = \
                (qkv_i[0][0], qkv_i[1][0]), (qkv_v[0][1], qkv_v[1][1]), \
                (qkv_v[0][2], qkv_v[1][2])
            wp, bp = inp['Wp_i'], inp['bp_i']
            lnw = (inp['ln1i_w'], inp['ln1i_b'], inp['ln2i_w'], inp['ln2i_b'])
            w1, b1, w2, b2 = inp['W1i'], inp['b1i'], inp['W2i'], inp['b2i']
        maps.append({
            'xq': np.ascontiguousarray(x_own[r0:r0 + T].T, np.float32),
            'xkv': np.ascontiguousarray(x_oth.T).astype(BF),
            'wq': wq.astype(BF), 'wk': wk.astype(BF), 'wv': wv.astype(BF),
            'wp': np.asarray(wp, np.float32).astype(BF),
            'w1': np.asarray(w1, np.float32).astype(BF),
            'w2': np.asarray(w2, np.float32).astype(BF),
            'bq': bq, 'bk': bk, 'bv': np.ascontiguousarray(bv[None, :]),
            'bp': np.asarray(bp, np.float32),
            'b1': np.asarray(b1, np.float32), 'b2': np.asarray(b2, np.float32),
            'ln1w': lnw[0], 'ln1b': lnw[1], 'ln2w': lnw[2], 'ln2b': lnw[3],
        })
    return maps


def _recon(x):
    x = x.reshape(14, 14, 16, 16)
    x = np.transpose(x, (2, 3, 0, 1))
    return x.reshape(1, 1, 224, 224)


def assemble(core_outs):
    ov = np.concatenate([core_outs[c].T for c in range(4)], axis=0)
    oi = np.concatenate([core_outs[c].T for c in range(4, 8)], axis=0)
    return np.concatenate([_recon(oi), _recon(ov)], axis=1).astype(np.float32)


def get_nc():
    if 'nc' not in _CACHE:
        _CACHE['nc'] = build_bass()
    return _CACHE['nc']


def kernel(**inputs):
    from concourse import bass_utils
    nc = get_nc()
    in_maps = make_in_maps(inputs)
    res = bass_utils.run_bass_kernel_spmd(nc, in_maps, core_ids=list(range(NCORES)))
    outs = [np.asarray(r['out'], np.float32) for r in res.results]
    return assemble(outs)


# revision 20
# speedup vs baseline: 1.2506x; 1.2506x over previous
"""CrossViT fused block on 8 TRN2 NeuronCores.

Sharding: 2 branches (vis-output / ir-output) x 4-way token split -> 8 cores,
no collectives. Each core computes 49 output tokens of one branch end-to-end:
LN1, cross-attention (its queries vs all 196 keys/values of the other
modality), projection, residual, LN2, FFN, residual. Activations are kept
feature-major (transposed: features on SBUF partitions) so every linear layer
is matmul(lhsT=W_natural, rhs=xT). Matmul operands are bf16 (fp32 PSUM
accumulation); LayerNorm / softmax statistics in fp32.

Engine budget: PE does all matmuls + transposes + partition
broadcasts/reductions; ACT only Exp/Sqrt/Gelu (grouped to avoid table
reloads); DVE all PSUM->SBUF copies and element-wise; GpSimd the softmax
normalize (normalize_recip). Inputs arrive as a few large contiguous
partition-major DMA blobs.

Host side: shards + head-major-reorders weights into the blobs, gathers the
8 (256, 49) outputs, applies the output pixel-shuffle.
"""
import sys
if '/opt/trn_rl_repo' not in sys.path:
    sys.path.insert(0, '/opt/trn_rl_repo')

import numpy as np
import ml_dtypes

BF = ml_dtypes.bfloat16
N, EMB, H, DH, HID = 196, 256, 8, 32, 1024
T = 49            # tokens per core
EPS, SCALE = 1e-5, 16.0
P = 128
NCORES = 8
TOKC = ((0, 128), (128, 68))   # token chunks of the 196 keys/values

_CACHE = {}


# ---------------------------------------------------------------- bass build
def build_bass():
    import concourse.bacc as bacc
    import concourse.mybir as mybir
    import concourse.tile as tile

    f32 = mybir.dt.float32
    bf16 = mybir.dt.bfloat16
    AF = mybir.ActivationFunctionType
    OP = mybir.AluOpType

    nc = bacc.Bacc("TRN2", target_bir_lowering=False)

    # ---- DRAM I/O: a few contiguous partition-major blobs
    xq_d = nc.dram_tensor("xq", [P, 2 * T], f32, kind="ExternalInput")
    xkv_d = nc.dram_tensor("xkv", [P, 2 * N], bf16, kind="ExternalInput")
    wqkv_d = nc.dram_tensor("wqkv", [P, 3 * 512], bf16, kind="ExternalInput")
    wpw1_d = nc.dram_tensor("wpw1", [P, 512 + 2048], bf16, kind="ExternalInput")
    w2_d = nc.dram_tensor("w2", [P, 2048], bf16, kind="ExternalInput")
    vec_d = nc.dram_tensor("vec", [P, 28], f32, kind="ExternalInput")
    bv_d = nc.dram_tensor("bv", [1, EMB], f32, kind="ExternalInput")
    out_d = nc.dram_tensor("out", [EMB, T], f32, kind="ExternalOutput")

    id98_d = nc.inline_tensor(np.eye(98, dtype=BF), name="id98c")

    with tile.TileContext(nc) as tc:
        with (
            tc.tile_pool(name="const", bufs=1) as cpool,
            tc.tile_pool(name="act", bufs=1) as apool,
            tc.tile_pool(name="attp", bufs=4) as attpool,
            tc.tile_pool(name="ps_mm", bufs=2, space="PSUM") as ps_mm,
            tc.tile_pool(name="ps_s", bufs=2, space="PSUM") as ps_s,
            tc.tile_pool(name="ps_t", bufs=2, space="PSUM") as ps_t,
            tc.tile_pool(name="ps_o", bufs=1, space="PSUM") as ps_o,
            tc.tile_pool(name="ps_ln", bufs=1, space="PSUM") as ps_ln,
        ):
            def load(dram_ap, shape, dt, tag):
                t = cpool.tile(shape, dt, tag=tag)
                nc.sync.dma_start(t[:], dram_ap)
                return t

            # DMAs in dependency order
            xq_sb = load(xq_d[:, :], [P, 2 * T], f32, "xq")
            xkv_sb = load(xkv_d[:, :], [P, 2 * N], bf16, "xkv")
            wqkv_sb = load(wqkv_d[:, :], [P, 3 * 512], bf16, "wqkv")
            vec_sb = load(vec_d[:, :], [P, 28], f32, "vec")
            bv_bc = load(bv_d[:, :].to_broadcast((P, EMB)), [P, EMB], f32, "bv")
            id98 = load(id98_d[:, :], [98, 98], bf16, "id98")
            wpw1_sb = load(wpw1_d[:, :], [P, 2560], bf16, "wpw1")
            w2_sb3 = load(w2_d[:, :], [P, 2048], bf16, "w2")

            # views
            wq_sb = wqkv_sb[:, 0:512].rearrange("p (c m) -> p c m", c=2)
            wk_sb = wqkv_sb[:, 512:1024].rearrange("p (c m) -> p c m", c=2)
            wv_sb = wqkv_sb[:, 1024:1536].rearrange("p (c m) -> p c m", c=2)
            wp_sb = wpw1_sb[:, 0:512].rearrange("p (c m) -> p c m", c=2)
            w1_sb = wpw1_sb[:, 512:2560].rearrange("p (c m) -> p c m", c=2)
            w2_sb = w2_sb3[:, :].rearrange("p (c m) -> p c m", c=8)
            xkv_bf = xkv_sb[:, :].rearrange("p (c t) -> p c t", c=2)
            # cols 0-3: bq head-pairs (rows 0-63); cols 4-7: bk head-pairs
            bq_v, bk_v = vec_sb[0:64, 0:4], vec_sb[0:64, 4:8]
            bp_v, b2_v = vec_sb[:, 8:10], vec_sb[:, 10:12]
            ln1w_v, ln1b_v = vec_sb[:, 12:14], vec_sb[:, 14:16]
            ln2w_v, ln2b_v = vec_sb[:, 16:18], vec_sb[:, 18:20]
            b1_v = vec_sb[:, 20:28]

            ones_bf = cpool.tile([P, 1], bf16, tag="ones_bf")
            nc.vector.memset(ones_bf[:], 1.0)
            ones1f = cpool.tile([1, P], f32, tag="ones1f")
            nc.vector.memset(ones1f[:], 1.0)
            eps_sb = cpool.tile([1, 1], f32, tag="eps")
            nc.vector.memset(eps_sb[:], EPS)

            xq3 = xq_sb[:, :].rearrange("p (c t) -> p c t", c=2)
            xq_bf = apool.tile([P, 2, T], bf16, tag="xq_bf")
            nc.vector.tensor_copy(xq_bf[:], xq3)

            def pair_bias(v, pr):
                # bias for head-pair pr: (64, 1) at partitions 0-63, lane-aligned
                # with the pair-major consumers
                return v[:, pr:pr + 1]

            # ---------------- q block-diag per pair, k pair-major, v token-major
            bd_tiles = []
            for pr in range(4):
                bd = attpool.tile([64, 2 * T], bf16, tag="bd")
                nc.vector.memset(bd[:], 0.0)
                for j in range(2):
                    h = 2 * pr + j
                    pbd = ps_t.tile([64, 512], f32, tag="small")
                    for kc in range(2):
                        nc.tensor.matmul(pbd[j * DH:(j + 1) * DH, j * T:(j + 1) * T],
                                         wq_sb[:, kc, h * DH:(h + 1) * DH],
                                         xq_bf[:, kc], start=(kc == 0), stop=(kc == 1))
                    bslc = pair_bias(bq_v, pr)[j * DH:(j + 1) * DH, :]
                    nc.vector.tensor_scalar(
                        bd[j * DH:(j + 1) * DH, j * T:(j + 1) * T],
                        pbd[j * DH:(j + 1) * DH, j * T:(j + 1) * T],
                        bslc, None, op0=OP.add)
                bd_tiles.append(bd)

            k_bf = apool.tile([64, 4, N], bf16, tag="k_bf")
            for pr in range(4):
                pk = ps_mm.tile([P, 512], f32, tag="mm")
                for kc in range(2):
                    nc.tensor.matmul(pk[:64, 0:N], wk_sb[:, kc, pr * 64:(pr + 1) * 64],
                                     xkv_bf[:, kc], start=(kc == 0), stop=(kc == 1))
                nc.vector.tensor_scalar(k_bf[:, pr], pk[:64, 0:N],
                                        pair_bias(bk_v, pr), None, op0=OP.add)

            v_bf = []
            for tcx, (t0, tsz) in enumerate(TOKC):
                pv = ps_mm.tile([P, 512], f32, tag="mm")
                for kc in range(2):
                    nc.tensor.matmul(pv[:tsz, 0:EMB], xkv_bf[:, kc, t0:t0 + tsz],
                                     wv_sb[:, kc], start=(kc == 0), stop=(kc == 1))
                vt = apool.tile([P, EMB], bf16, tag=f"v{tcx}")
                nc.vector.tensor_tensor(vt[:tsz], pv[:tsz, 0:EMB], bv_bc[:tsz],
                                        op=OP.add)
                v_bf.append(vt)

            # ---------------- scores + softmax, pipelined per head pair
            ssum = apool.tile([2 * T, 4], f32, tag="ssum")
            attn_tiles = []
            for pr in range(4):
                pss = ps_s.tile([2 * T, 512], f32, tag="scores")
                nc.tensor.matmul(pss[:, 0:N], bd_tiles[pr][:], k_bf[:, pr],
                                 start=True, stop=True)
                atf = attpool.tile([2 * T, N], f32, tag="attf")
                nc.scalar.activation(atf[:], pss[:, 0:N], AF.Exp, scale=1.0 / SCALE,
                                     accum_out=ssum[:, pr:pr + 1])
                att = attpool.tile([2 * T, N], bf16, tag="attn")
                nc.gpsimd.normalize_recip(att[:], atf[:], ssum[:, pr:pr + 1])
                attn_tiles.append(att)

            # ---------------- attT via PE transpose, then o^T_h = v_h^T @ att_h^T
            po = ps_o.tile([P, 512], f32, tag="oT")
            for pr in range(4):
                ats = []
                for tcx, (t0, tsz) in enumerate(TOKC):
                    pt = ps_t.tile([P, 1024], bf16, tag="small")
                    nc.tensor.transpose(pt[:tsz, 0:2 * T],
                                        attn_tiles[pr][:, t0:t0 + tsz], id98[:])
                    at = attpool.tile([P, 2 * T], bf16, tag="attT")
                    nc.vector.tensor_copy(at[:tsz], pt[:tsz, 0:2 * T])
                    ats.append(at)
                for j in range(2):
                    h = 2 * pr + j
                    mc, prow = h // 4, (h % 4) * DH
                    for tcx, (t0, tsz) in enumerate(TOKC):
                        nc.tensor.matmul(po[prow:prow + DH, mc * T:(mc + 1) * T],
                                         v_bf[tcx][:tsz, h * DH:(h + 1) * DH],
                                         ats[tcx][:tsz, j * T:(j + 1) * T],
                                         start=(tcx == 0), stop=(tcx == 1),
                                         tile_position=(0, prow))

            oT_bf = apool.tile([P, 2, T], bf16, tag="oT_bf")
            for mc in range(2):
                nc.vector.tensor_copy(oT_bf[:, mc], po[:, mc * T:(mc + 1) * T])

            # ---------------- layernorm (feature-major: stats over partitions)
            # ACT is used only for Sqrt (both LNs' Sqrts are adjacent in ACT
            # program order: LN1 emitted after the exps, LN2 before the gelus).
            def layer_norm(x_f32, x_bf, w_v, b_v, out_f32, tag):
                lnp = ps_ln.tile([P, 512], f32, tag="ln")
                xsq = apool.tile([P, 2, 2 * T], bf16, tag=f"xsq_{tag}")
                nc.vector.tensor_copy(xsq[:, :, 0:T], x_bf[:])
                nc.vector.tensor_tensor(xsq[:, :, T:2 * T], x_bf[:], x_bf[:],
                                        op=OP.mult)
                for kc in range(2):
                    nc.tensor.matmul(lnp[0:1, 0:2 * T], ones_bf[:], xsq[:, kc],
                                     start=(kc == 0), stop=(kc == 1))
                # mr: [0:T] mean, [T:2T] rstd
                mr = apool.tile([1, 2 * T], f32, tag=f"mr_{tag}")
                nc.vector.tensor_scalar(mr[0:1, 0:T], lnp[0:1, 0:T], 1.0 / EMB,
                                        None, op0=OP.mult)
                m2 = apool.tile([1, T], f32, tag=f"m2_{tag}")
                nc.vector.tensor_tensor(m2[:], mr[0:1, 0:T], mr[0:1, 0:T],
                                        op=OP.mult)
                var = apool.tile([1, T], f32, tag=f"var_{tag}")
                nc.vector.tensor_scalar(var[:], lnp[0:1, T:2 * T], 1.0 / EMB, None,
                                        op0=OP.mult)
                nc.vector.tensor_tensor(var[:], var[:], m2[:], op=OP.subtract)
                std = apool.tile([1, T], f32, tag=f"std_{tag}")
                nc.scalar.activation(std[:], var[:], AF.Sqrt, bias=eps_sb[:])
                nc.vector.reciprocal(mr[0:1, T:2 * T], std[:])
                nc.tensor.matmul(lnp[:, 2 * T:4 * T], ones1f[:], mr[:],
                                 start=True, stop=True)
                tmp = apool.tile([P, 2, T], f32, tag=f"lntmp_{tag}")
                for kc in range(2):
                    nc.vector.tensor_tensor(tmp[:, kc], x_f32[:, kc],
                                            lnp[:, 2 * T:3 * T], op=OP.subtract)
                    nc.vector.tensor_tensor(tmp[:, kc], tmp[:, kc],
                                            lnp[:, 3 * T:4 * T], op=OP.mult)
                    nc.vector.tensor_scalar(out_f32[:, kc], tmp[:, kc],
                                            w_v[:, kc:kc + 1], b_v[:, kc:kc + 1],
                                            op0=OP.mult, op1=OP.add)

            nvT = apool.tile([P, 2, T], f32, tag="nvT")
            layer_norm(xq3, xq_bf, ln1w_v, ln1b_v, nvT, "ln1")

            # ---------------- projection + residual
            rv = apool.tile([P, 2, T], f32, tag="rv")
            for mc in range(2):
                pp = ps_mm.tile([P, 512], f32, tag="mm")
                for kc in range(2):
                    nc.tensor.matmul(pp[:, 0:T], wp_sb[:, kc, mc * P:(mc + 1) * P],
                                     oT_bf[:, kc], start=(kc == 0), stop=(kc == 1))
                nc.vector.tensor_scalar(rv[:, mc], pp[:, 0:T], bp_v[:, mc:mc + 1],
                                        None, op0=OP.add)
                nc.vector.tensor_tensor(rv[:, mc], rv[:, mc], nvT[:, mc], op=OP.add)
            rv_bf = apool.tile([P, 2, T], bf16, tag="rvbf")
            nc.vector.tensor_copy(rv_bf[:], rv[:])

            # ---------------- LN2
            lv = apool.tile([P, 2, T], f32, tag="lv")
            layer_norm(rv, rv_bf, ln2w_v, ln2b_v, lv, "ln2")
            lv_bf = apool.tile([P, 2, T], bf16, tag="lvbf")
            nc.vector.tensor_copy(lv_bf[:], lv[:])

            # ---------------- FFN + residual
            g_bf = apool.tile([P, 8, T], bf16, tag="gelu")
            for mc in range(8):
                ph = ps_mm.tile([P, 512], f32, tag="mm")
                for kc in range(2):
                    nc.tensor.matmul(ph[:, 0:T], w1_sb[:, kc, mc * P:(mc + 1) * P],
                                     lv_bf[:, kc], start=(kc == 0), stop=(kc == 1))
                nc.scalar.activation(g_bf[:, mc], ph[:, 0:T], AF.Gelu,
                                     bias=b1_v[:, mc:mc + 1])

            out_sb = apool.tile([P, 2, T], f32, tag="out")
            for mc in range(2):
                pf = ps_mm.tile([P, 512], f32, tag="mm")
                for kc in range(8):
                    nc.tensor.matmul(pf[:, 0:T], w2_sb[:, kc, mc * P:(mc + 1) * P],
                                     g_bf[:, kc], start=(kc == 0), stop=(kc == 7))
                nc.vector.tensor_scalar(out_sb[:, mc], pf[:, 0:T], b2_v[:, mc:mc + 1],
                                        None, op0=OP.add)
                nc.vector.tensor_tensor(out_sb[:, mc], out_sb[:, mc], lv[:, mc],
                                        op=OP.add)

            nc.sync.dma_start(out_d.rearrange("(c p) t -> p c t", p=P), out_sb[:])

    nc.compile()
    return nc


# ---------------------------------------------------------------- host side
def _reorder_qkv(W, b):
    W4 = np.asarray(W, np.float32).reshape(EMB, H, DH, 3)
    b4 = np.asarray(b, np.float32).reshape(H, DH, 3)
    return ([np.ascontiguousarray(W4[:, :, :, i].reshape(EMB, EMB)) for i in range(3)],
            [np.ascontiguousarray(b4[:, :, i].reshape(EMB)) for i in range(3)])


def _pack_w(w):
    """(K, M) f32 -> partition-major (128, K//128 * M) bf16 blob block."""
    w = np.asarray(w, np.float32)
    k, m = w.shape
    c = k // P
    return np.transpose(w.reshape(c, P, m), (1, 0, 2)).reshape(P, c * m).astype(BF)


def _pack_v(v):
    v = np.asarray(v, np.float32)
    c = v.shape[0] // P
    return np.ascontiguousarray(v.reshape(c, P).T)


def _pack_x(x):
    """(tokens, 256) -> (128, 2*tokens) f32 partition-major transposed."""
    xt = np.ascontiguousarray(np.asarray(x, np.float32).T)       # (256, t)
    t = xt.shape[1]
    return np.transpose(xt.reshape(2, P, t), (1, 0, 2)).reshape(P, 2 * t)


def make_in_maps(inputs):
    inp = {k: np.asarray(v, np.float32) for k, v in inputs.items()}
    qkv_v = _reorder_qkv(inp['Wqkv_v'], inp['bqkv_v'])
    qkv_i = _reorder_qkv(inp['Wqkv_i'], inp['bqkv_i'])
    maps = []
    for core in range(NCORES):
        branch = core // 4
        r0 = (core % 4) * T
        if branch == 0:   # vis output: vis queries, ir keys/values
            x_own, x_oth = inp['vis_emb'][0], inp['ir_emb'][0]
            wq, bq = qkv_v[0][0], qkv_v[1][0]
            wk, bk = qkv_i[0][1], qkv_i[1][1]
            wv, bv = qkv_i[0][2], qkv_i[1][2]
            wp, bp = inp['Wp_v'], inp['bp_v']
            lnw = (inp['ln1v_w'], inp['ln1v_b'], inp['ln2v_w'], inp['ln2v_b'])
            w1, b1, w2, b2 = inp['W1v'], inp['b1v'], inp['W2v'], inp['b2v']
        else:             # ir output: ir queries, vis keys/values
            x_own, x_oth = inp['ir_emb'][0], inp['vis_emb'][0]
            wq, bq = qkv_i[0][0], qkv_i[1][0]
            wk, bk = qkv_v[0][1], qkv_v[1][1]
            wv, bv = qkv_v[0][2], qkv_v[1][2]
            wp, bp = inp['Wp_i'], inp['bp_i']
            lnw = (inp['ln1i_w'], inp['ln1i_b'], inp['ln2i_w'], inp['ln2i_b'])
            w1, b1, w2, b2 = inp['W1i'], inp['b1i'], inp['W2i'], inp['b2i']
        # bq/bk as (64, 4) head-pair columns, zero-padded to 128 rows
        def pairs(v):
            q = np.zeros((P, 4), np.float32)
            q[0:64, :] = np.asarray(v, np.float32).reshape(4, 64).T
            return q
        vec = np.concatenate([
            pairs(bq), pairs(bk), _pack_v(np.asarray(bp, np.float32)),
            _pack_v(np.asarray(b2, np.float32)),
            _pack_v(lnw[0]), _pack_v(lnw[1]), _pack_v(lnw[2]), _pack_v(lnw[3]),
            _pack_v(np.asarray(b1, np.float32)),
        ], axis=1).astype(np.float32)
        maps.append({
            'xq': _pack_x(x_own[r0:r0 + T]),
            'xkv': _pack_x(x_oth).astype(BF),
            'wqkv': np.concatenate([_pack_w(wq), _pack_w(wk), _pack_w(wv)], axis=1),
            'wpw1': np.concatenate([_pack_w(wp), _pack_w(w1)], axis=1),
            'w2': _pack_w(w2),
            'vec': np.ascontiguousarray(vec),
            'bv': np.ascontiguousarray(bv[None, :]).astype(np.float32),
        })
    return maps


def _recon(x):
    x = x.reshape(14, 14, 16, 16)
    x = np.transpose(x, (2, 3, 0, 1))
    return x.reshape(1, 1, 224, 224)


def assemble(core_outs):
    ov = np.concatenate([core_outs[c].T for c in range(4)], axis=0)
    oi = np.concatenate([core_outs[c].T for c in range(4, 8)], axis=0)
    return np.concatenate([_recon(oi), _recon(ov)], axis=1).astype(np.float32)


def get_nc():
    if 'nc' not in _CACHE:
        _CACHE['nc'] = build_bass()
    return _CACHE['nc']


def kernel(**inputs):
    from concourse import bass_utils
    nc = get_nc()
    in_maps = make_in_maps(inputs)
    res = bass_utils.run_bass_kernel_spmd(nc, in_maps, core_ids=list(range(NCORES)))
    outs = [np.asarray(r['out'], np.float32) for r in res.results]
    return assemble(outs)
